# revision 2
# baseline (speedup 1.0000x reference)
"""ComplEx rhs-scoring kernel for Trainium2 (8 NeuronCores).

scores = Re(<lhs * rel, conj(all_ents)>) = q @ ent_emb.T
where q = [q_re, q_im] (complex product of gathered lhs/rel embeddings).

Strategy (tensor-parallel over candidates):
  - host: gather + complex product -> q [B, K] (tiny, exact fp32),
    transpose to qT [K, B]; transpose ent_emb -> eT [K, N]; split eT
    into 8 column slabs [K, N/8] (one per core); replicate qT.
  - device (per core): scores_slab[b, n] = sum_k qT[k, b] * eT[k, n]
    via PE matmuls: lhsT = qT k-tile [128, 128], rhs = eT chunk
    [128, CW], accumulate K/128 = 8 matmuls into PSUM fp32.
  - host: concat slabs along axis 1 -> [B, N] (upcast bf16 -> f32).

The PE stream floor is 64 matmuls/chunk-col * 12500 cols = 800k
cycles = 333.3us at 2.4GHz; everything else is overlap + teardown
engineering.  The teardown (per-semaphore clears, ~115ns each on the
tensor queue) scales with the number of distinct semaphores the
program uses, so the kernel batches DMAs (1 per et chunk, writebacks
2 chunks at a time) and pairs PSUM banks ([P,1024] tiles -> 4 copies
per chunk instead of 8) to keep the semaphore count low.
"""

import os
import numpy as np

import concourse.bacc as bacc
import concourse.mybir as mybir
import concourse.tile as tile
from concourse.bass_utils import run_bass_kernel_spmd

N_CORES = 8
B = 1024          # batch (queries)
K = 1024          # contraction dim (2 * rank)
N_ENT = 100000    # candidates
NS = N_ENT // N_CORES  # per-core slab width (12500)
P = 128           # partitions
KT = K // P       # k tiles (8)
BT = B // P       # b tiles (8)
W_LAST = 212      # final (drain) chunk width
N_MAIN = NS - W_LAST  # 12288

_DT = {
    "bf16": mybir.dt.bfloat16,
    "f32r": mybir.dt.float32r,
    "f32": mybir.dt.float32,
}


def build_kernel(dt_name, ns=NS, b=B):
    dt_in = _DT[dt_name]
    f32 = mybir.dt.float32
    dt_out = dt_in if dt_name == "bf16" else f32
    nc = bacc.Bacc("TRN2", target_bir_lowering=False, debug=False)

    qT = nc.dram_tensor("qT", [K, b], dt_in, kind="ExternalInput")
    eT = nc.dram_tensor("eT", [K, ns], dt_in, kind="ExternalInput")
    # main chunks go to `out`; the final 212-wide drain chunk is a RAW
    # tile dump to out2 (contiguous per partition -> coarse DMA
    # descriptors); the host interleaves it back.
    out = nc.dram_tensor("out", [b, N_MAIN], dt_out, kind="ExternalOutput")
    bt = b // P
    out2 = nc.dram_tensor("out2", [P, bt * W_LAST], dt_out,
                          kind="ExternalOutput")
    out2_r = out2.rearrange("p (bt w) -> p bt w", bt=bt)

    # chunk widths: small first chunk (256) so the head's first et
    # slice lands early; small last chunk (212) so the post-stream
    # drain (copy -> dma issue -> transfer) trails the final matmul
    # minimally.  256+23*512+256 = 12288 main + 212 drain = 12500.
    widths = [256] + [512] * 23 + [256, 212]
    offs = [sum(widths[:i]) for i in range(len(widths))]
    n_chunks = len(widths)
    w0 = widths[0]

    # 3D-AP views: partition dim first, k/b tile index as a middle dim
    # so a whole chunk moves in ONE dma_start.
    eT_r = eT.rearrange("(kt p) n -> p kt n", p=P)    # [128, KT, ns]
    qT_r = qT.rearrange("(kt p) b -> p kt b", p=P)    # [128, KT, b]
    out_r = out.rearrange("(bt p) n -> p bt n", p=P)  # [128, bt, n_main]

    # writeback groups: pairs of main chunks share one SBUF out tile
    # and one coarse DMA.  [(0,1),(2,3),...,(22,23),(24,)]
    wb_groups = [(c, c + 1) for c in range(0, 24, 2)] + [(24,)]
    group_of = {}
    for g, grp in enumerate(wb_groups):
        for c in grp:
            group_of[c] = g

    with tile.TileContext(nc) as tc:
        with (
            tc.tile_pool(name="qpool", bufs=1) as qpool,
            tc.tile_pool(name="epool", bufs=4) as epool,
            tc.tile_pool(name="pspool", bufs=4, space="PSUM") as pspool,
            tc.tile_pool(name="opool", bufs=2) as opool,
        ):
            et0 = epool.tile([P, KT * w0], dt_in, tag="et")
            qsb = qpool.tile([P, KT * b], dt_in)
            qsb_r = qsb.rearrange("p (kt b) -> p kt b", kt=KT)

            # PE warmup (HAM clock-gate): thin dummy matmuls on a
            # memset tile keep the PE busy from right after the
            # preamble until the first real data lands.  128-wide so
            # they drain fast once data is ready (each ~107ns cold);
            # 20 of them cover ~2.1us of head DMA wait, and a late
            # data arrival leaves only a short (<3.4us) idle gap that
            # does not re-cool the clock.
            ww = 128
            warm = qpool.tile([P, ww], mybir.dt.bfloat16, name="warm")
            nc.vector.memset(warm[:], 0.0)
            ps_w = pspool.tile([P, 2 * ww], f32, tag="ps", name="ps_warm")
            for _ in range(20):
                nc.tensor.matmul(ps_w[:, 0:ww], warm[:], warm[:],
                                 start=True, stop=True)

            # head loads, k-granular so compute can start after just
            # q[k0 quarter]+et0[k0] (~128KB) instead of the full 2.5MB.
            # q k0 goes in 4 b-quarters alternating sync/gpsimd (the
            # chunk-0 k-major loop consumes b-tiles in order); later k
            # slices alternate sync/gpsimd; et0 k-slices ride scalar.
            bq = b // 4
            nc.sync.dma_start(qsb_r[:, 0, 0:bq], qT_r[:, 0, 0:bq])
            nc.gpsimd.dma_start(qsb_r[:, 0, bq:2 * bq],
                                qT_r[:, 0, bq:2 * bq])
            nc.sync.dma_start(qsb_r[:, 0, 2 * bq:3 * bq],
                              qT_r[:, 0, 2 * bq:3 * bq])
            nc.gpsimd.dma_start(qsb_r[:, 0, 3 * bq:b],
                                qT_r[:, 0, 3 * bq:b])
            nc.scalar.dma_start(et0[:, 0:w0], eT[0:P, 0:w0])
            for k in range(1, KT):
                eng = nc.sync if k % 2 else nc.gpsimd
                eng.dma_start(qsb_r[:, k, :], qT_r[:, k, :])
                nc.scalar.dma_start(
                    et0[:, k * w0:(k + 1) * w0],
                    eT[k * P:(k + 1) * P, 0:w0],
                )

            ot = None
            for c in range(n_chunks):
                w = widths[c]
                off = offs[c]
                last = c == n_chunks - 1
                if c == 0:
                    et = et0
                else:
                    et = epool.tile([P, KT * w], dt_in, tag="et",
                                    name=f"et{c}")
                    et_v = et.rearrange("p (kt w) -> p kt w", kt=KT)
                    if c == 1:
                        # chunk 1 races the head q loads: split halves
                        # across scalar (done with et0 early) and sync
                        # (busy with q until ~15us)
                        kh = KT // 2
                        nc.scalar.dma_start(
                            et_v[:, 0:kh, :],
                            eT_r[:, 0:kh, off:off + w])
                        nc.sync.dma_start(
                            et_v[:, kh:KT, :],
                            eT_r[:, kh:KT, off:off + w])
                    else:
                        nc.sync.dma_start(
                            et_v[:, :, :],
                            eT_r[:, :, off:off + w])

                if last:
                    # drain chunk: unpaired bi-major, per-bi copies and
                    # three writeback slices so only bi7's copy + a
                    # 54KB DMA trail the final matmul.
                    oL = opool.tile([P, bt * w], dt_out, tag="ot",
                                    name="ot_last")
                    oL_h = oL.rearrange("p (bt w) -> p bt w", bt=bt)
                    for bi in range(bt):
                        ps = pspool.tile([P, w], f32, tag="ps",
                                         name="ps_l")
                        for k in range(KT):
                            nc.tensor.matmul(
                                ps[:],
                                qsb[:, k * b + bi * P:k * b + (bi + 1) * P],
                                et[:, k * w:(k + 1) * w],
                                start=(k == 0),
                                stop=(k == KT - 1),
                            )
                        eng = nc.vector.tensor_copy if bi % 2 else nc.scalar.copy
                        eng(oL[:, bi * w:(bi + 1) * w], ps[:])
                        if bi == 3:
                            nc.scalar.dma_start(
                                out2_r[:, 0:4, :], oL_h[:, 0:4, :])
                        elif bi == 6:
                            nc.scalar.dma_start(
                                out2_r[:, 4:7, :], oL_h[:, 4:7, :])
                        elif bi == 7:
                            nc.scalar.dma_start(
                                out2_r[:, 7:8, :], oL_h[:, 7:8, :])
                    continue

                # output tile for this chunk's writeback group
                g = group_of[c]
                grp = wb_groups[g]
                if c == grp[0]:
                    gw = sum(widths[cc] for cc in grp)
                    goff = offs[grp[0]]
                    ot = opool.tile([P, bt * gw], dt_out, tag="ot",
                                    name=f"ot{g}")
                    ot_h = ot.rearrange("p (bt w) -> p bt w", bt=bt)
                coff = off - goff  # this chunk's column base inside ot

                # paired PSUM tiles: [P, 1024] f32 spans 2 banks; even
                # bi accumulates in bank A ([0:w]), odd bi in bank B
                # ([512:512+w]) so accumulation groups stay bank
                # aligned.  One (possibly strided) copy per pair.
                pairs = [pspool.tile([P, 1024], f32, tag="ps",
                                     name=f"ps{c}_{p}") for p in range(4)]
                if c == 0:
                    # k-major: all 8 b-tiles accumulate simultaneously;
                    # each k-step needs only q[k]+et0[k] so compute
                    # starts right off the first quarter-DMAs.
                    for k in range(KT):
                        for bi in range(bt):
                            ps = pairs[bi // 2]
                            col = (bi % 2) * 512
                            nc.tensor.matmul(
                                ps[:, col:col + w],
                                qsb[:, k * b + bi * P:k * b + (bi + 1) * P],
                                et[:, k * w:(k + 1) * w],
                                start=(k == 0),
                                stop=(k == KT - 1),
                            )
                else:
                    # k-half-major: run all b-tiles over k0-3 first,
                    # then k4-7, so chunk 1 can start on its first
                    # half-DMA before the second lands.
                    for h in range(2):
                        for bi in range(bt):
                            ps = pairs[bi // 2]
                            col = (bi % 2) * 512
                            for kk in range(4):
                                k = 4 * h + kk
                                nc.tensor.matmul(
                                    ps[:, col:col + w],
                                    qsb[:, k * b + bi * P:k * b + (bi + 1) * P],
                                    et[:, k * w:(k + 1) * w],
                                    start=(k == 0),
                                    stop=(k == KT - 1),
                                )
                for p in range(4):
                    src = pairs[p].rearrange("q (two x) -> q two x", two=2)
                    dst = ot_h[:, 2 * p:2 * p + 2, coff:coff + w]
                    if p % 2 == 0:
                        nc.vector.tensor_copy(dst, src[:, :, 0:w])
                    else:
                        nc.scalar.copy(dst, src[:, :, 0:w])

                if c == grp[-1]:
                    nc.scalar.dma_start(
                        out_r[:, :, goff:goff + gw], ot_h[:, :, :])
    nc.compile()
    return nc


def _prep_inputs(x, ent_emb, rel_emb, dt_name):
    x = np.asarray(x)
    ent_emb = np.asarray(ent_emb, dtype=np.float32)
    rel_emb = np.asarray(rel_emb, dtype=np.float32)
    r = ent_emb.shape[1] // 2
    lhs = ent_emb[x[:, 0]]
    rel = rel_emb[x[:, 1]]
    lre, lim = lhs[:, :r], lhs[:, r:]
    rre, rim = rel[:, :r], rel[:, r:]
    q = np.empty((x.shape[0], 2 * r), np.float32)
    q[:, :r] = lre * rre - lim * rim
    q[:, r:] = lre * rim + lim * rre

    if dt_name == "bf16":
        import ml_dtypes
        np_dt = ml_dtypes.bfloat16
    else:
        np_dt = np.float32

    qT = np.ascontiguousarray(q.T).astype(np_dt)           # [K, B]
    eT = np.ascontiguousarray(ent_emb.T).astype(np_dt)     # [K, N]
    in_maps = [
        {"qT": qT, "eT": np.ascontiguousarray(eT[:, i * NS:(i + 1) * NS])}
        for i in range(N_CORES)
    ]
    return in_maps


def run(x, ent_emb, rel_emb, dt_name=None, trace=False, **spmd_kwargs):
    dt_name = dt_name or os.environ.get("KERNEL_DT", "bf16")
    nc = build_kernel(dt_name)
    in_maps = _prep_inputs(x, ent_emb, rel_emb, dt_name)
    res = run_bass_kernel_spmd(
        nc, in_maps, list(range(N_CORES)), trace=trace, **spmd_kwargs
    )
    outs = []
    for i in range(N_CORES):
        main = np.asarray(res.results[i]["out"], dtype=np.float32)
        tail = np.asarray(res.results[i]["out2"], dtype=np.float32)
        # out2 is a raw [P, bt*w_last] tile dump; row bt*P+p of the
        # slab is tail[p, bt*w_last:(bt+1)*w_last]
        tail = tail.reshape(P, BT, W_LAST).transpose(1, 0, 2).reshape(
            B, W_LAST)
        outs.append(main)
        outs.append(tail)
    return np.concatenate(outs, axis=1), res


def kernel(x, ent_emb, rel_emb):
    out, _ = run(x, ent_emb, rel_emb)
    return out


# revision 3
# speedup vs baseline: 1.0041x; 1.0041x over previous
"""ComplEx rhs-scoring kernel for Trainium2 (8 NeuronCores).

scores = Re(<lhs * rel, conj(all_ents)>) = q @ ent_emb.T
where q = [q_re, q_im] (complex product of gathered lhs/rel embeddings).

Strategy (tensor-parallel over candidates):
  - host: gather + complex product -> q [B, K] (tiny, exact fp32),
    transpose to qT [K, B]; transpose ent_emb -> eT [K, N]; split eT
    into 8 column slabs [K, N/8] (one per core); replicate qT.
  - device (per core): scores_slab[b, n] = sum_k qT[k, b] * eT[k, n]
    via PE matmuls: lhsT = qT k-tile [128, 128], rhs = eT chunk
    [128, CW], accumulate K/128 = 8 matmuls into PSUM fp32.
  - host: concat slabs along axis 1 -> [B, N] (upcast bf16 -> f32).

Timing structure (per core, bf16): 1600 matmuls (24x512 + 1x212
chunks, x8 b-tiles x8 k-tiles) at the N/2.4GHz streaming floor
= 333us; everything else (head DMA latency, PE clock-gate warmup,
output drain, framework pre/postamble) is overlap engineering around
that floor. bf16 (vs f32r) matters twice: FWL halves LDWEIGHTS so it
fully hides under the 211ns matmul stream (f32r measured 227ns/MM),
and input DMA halves.
"""

import os
import numpy as np

import concourse.bacc as bacc
import concourse.mybir as mybir
import concourse.tile as tile
from concourse.bass_utils import run_bass_kernel_spmd

N_CORES = 8
B = 1024          # batch (queries)
K = 1024          # contraction dim (2 * rank)
N_ENT = 100000    # candidates
NS = N_ENT // N_CORES  # per-core slab width (12500)
P = 128           # partitions
KT = K // P       # k tiles (8)
BT = B // P       # b tiles (8)
CW = 512          # rhs chunk width (one full PSUM bank)

_DT = {
    "bf16": mybir.dt.bfloat16,
    "f32r": mybir.dt.float32r,
    "f32": mybir.dt.float32,
}


def build_kernel(dt_name, ns=NS, cw=CW, b=B):
    dt_in = _DT[dt_name]
    f32 = mybir.dt.float32
    # bf16 path also writes bf16 output (host upcasts): halves the
    # writeback DMA traffic; rounding adds <0.1% error vs the 2% gate
    dt_out = dt_in if dt_name == "bf16" else f32
    nc = bacc.Bacc("TRN2", target_bir_lowering=False, debug=False)

    qT = nc.dram_tensor("qT", [K, b], dt_in, kind="ExternalInput")
    eT = nc.dram_tensor("eT", [K, ns], dt_in, kind="ExternalInput")
    # the last (remainder) chunk is written as a RAW tile dump to out2
    # (contiguous per partition -> coarse DMA descriptors, ~3x faster
    # completion than the strided [b, ns] write); the host interleaves
    # it back. Everything else goes to out.
    w_last = ns % cw if ns % cw else cw
    n_main = ns - w_last
    out = nc.dram_tensor("out", [b, n_main], dt_out, kind="ExternalOutput")
    bt_ = b // P
    out2 = nc.dram_tensor("out2", [P, bt_ * w_last], dt_out,
                          kind="ExternalOutput")
    out2_r = out2.rearrange("p (bt w) -> p bt w", bt=bt_)

    bt = b // P
    # remainder chunk goes LAST: its copies and writeback are ~2.4x
    # smaller than a full chunk's, so the post-stream drain (which is
    # serial: last matmul -> copy -> dma issue -> transfer -> teardown)
    # trails the last matmul minimally
    widths = [cw] * (ns // cw)
    if ns % cw:
        widths = widths + [ns % cw]
    offs = [sum(widths[:i]) for i in range(len(widths))]
    n_chunks = len(widths)
    w0 = widths[0]

    # 3D-AP views: put the 128-partition dim first, keep k/b tile index
    # as a middle dim so a whole chunk moves in ONE dma_start (the sync
    # engine's ~0.7us per-issue cost is the scarce resource here).
    eT_r = eT.rearrange("(kt p) n -> p kt n", p=P)    # [128, KT, ns]
    qT_r = qT.rearrange("(kt p) b -> p kt b", p=P)    # [128, KT, b]
    out_r = out.rearrange("(bt p) n -> p bt n", p=P)  # [128, bt, n_main]

    with tile.TileContext(nc) as tc:
        with (
            tc.tile_pool(name="qpool", bufs=1) as qpool,
            tc.tile_pool(name="epool", bufs=4) as epool,
            tc.tile_pool(name="pspool", bufs=8, space="PSUM") as pspool,
            tc.tile_pool(name="opool", bufs=2) as opool,
        ):
            et0 = epool.tile([P, KT * w0], dt_in, tag="et")
            qsb = qpool.tile([P, KT * b], dt_in)
            qsb_r = qsb.rearrange("p (kt b) -> p kt b", kt=KT)

            kh = KT // 2

            # PE warmup (HAM clock-gate): a few dummy matmuls on a
            # memset tile keep the PE busy from right after the
            # preamble until the first real data lands; chunk 0's own
            # cold-rate matmuls then finish the ~3.4us warm window
            # doing real work. gpsimd executes the memset because its
            # queue frees first after the framework preamble.
            ww = 250
            warm = qpool.tile([P, ww], mybir.dt.bfloat16, name="warm")
            nc.gpsimd.memset(warm[:], 0.0)
            ps_w = pspool.tile([P, ww], f32, tag="ps", name="ps_warm")
            # 16 bridges head-DMA jitter: when data lands late (~10.7us)
            # a shorter bridge leaves a PE idle gap and the clock gate
            # re-cools (measured +2.5us on bad runs); when data is early
            # the extra warmups cost at most ~0.4us of queue delay
            for _ in range(16):
                nc.tensor.matmul(ps_w[:], warm[:, 0:P], warm[:],
                                 start=True, stop=True)

            # head loads, k-granular so compute can start after just
            # q[k0]+et0[k0] (~0.4MB) instead of the full 3MB. The
            # ~0.7us per-dma_start issue cost is per-QUEUE, so the
            # first k-slice fans across the three DMA-capable queues
            # (sync+gpsimd for q halves, scalar for et0); remaining
            # k-slices alternate sync (q) / scalar (et0).
            bh = b // 2
            nc.sync.dma_start(qsb_r[:, 0, 0:bh], qT_r[:, 0, 0:bh])
            nc.gpsimd.dma_start(qsb_r[:, 0, bh:b], qT_r[:, 0, bh:b])
            nc.scalar.dma_start(et0[:, 0:w0], eT[0:P, 0:w0])
            for k in range(1, KT):
                nc.sync.dma_start(qsb_r[:, k, :], qT_r[:, k, :])
                nc.scalar.dma_start(
                    et0[:, k * w0:(k + 1) * w0],
                    eT[k * P:(k + 1) * P, 0:w0],
                )

            for c in range(n_chunks):
                w = widths[c]
                off = offs[c]
                if c == 0:
                    et = et0
                else:
                    et = epool.tile([P, KT * w], dt_in, tag="et", name=f"et{c}")
                    et_v = et.rearrange("p (kt w) -> p kt w", kt=KT)
                    # both k-halves stay on sync: queue order naturally
                    # deprioritizes this prefetch behind the head's
                    # critical q loads (splitting onto gpsimd made the
                    # prefetch compete with the head and stalled chunk 0)
                    for j in range(2):
                        nc.sync.dma_start(
                            et_v[:, j * kh:(j + 1) * kh, :],
                            eT_r[:, j * kh:(j + 1) * kh, off:off + w],
                        )
                ot = opool.tile([P, bt * w], dt_out, tag="ot", name=f"ot{c}")
                ot_h = ot.rearrange("p (bt w) -> p bt w", bt=bt)

                if c == 0:
                    # k-major: all 8 b-tiles accumulate in 8 PSUM banks
                    # simultaneously; each k-step needs only q[k]+et0[k]
                    # so compute starts ~7us in, right off the first DMA
                    pss = [pspool.tile([P, w], f32, tag="ps",
                                       name=f"ps0_{bi}") for bi in range(bt)]
                    for k in range(KT):
                        for bi in range(bt):
                            nc.tensor.matmul(
                                pss[bi][:],
                                qsb[:, k * b + bi * P:k * b + (bi + 1) * P],
                                et[:, k * w:(k + 1) * w],
                                start=(k == 0),
                                stop=(k == KT - 1),
                            )
                    for bi in range(bt):
                        if bi % 2 == 0:
                            nc.vector.tensor_copy(
                                ot[:, bi * w:(bi + 1) * w], pss[bi][:])
                        else:
                            nc.scalar.copy(
                                ot[:, bi * w:(bi + 1) * w], pss[bi][:])
                else:
                    # bi-major: one PSUM bank at a time, k inner
                    for bi in range(bt):
                        ps = pspool.tile([P, w], f32, tag="ps", name="ps")
                        for k in range(KT):
                            nc.tensor.matmul(
                                ps[:],
                                qsb[:, k * b + bi * P:k * b + (bi + 1) * P],
                                et[:, k * w:(k + 1) * w],
                                start=(k == 0),
                                stop=(k == KT - 1),
                            )
                        if c == n_chunks - 1:
                            # drain chunk: bi7's copy rides the vector
                            # engine (~0.2us vs scalar ACT 0.44us); the
                            # writeback goes out in three slices so only
                            # a 54KB DMA trails the final matmul, and
                            # that last issue rides the otherwise-idle
                            # sync queue.
                            if bi == 7:
                                nc.vector.tensor_copy(
                                    ot[:, bi * w:(bi + 1) * w], ps[:])
                                nc.sync.dma_start(
                                    out2_r[:, 7:8, :], ot_h[:, 7:8, :])
                            else:
                                if bi % 2 == 0:
                                    nc.vector.tensor_copy(
                                        ot[:, bi * w:(bi + 1) * w], ps[:])
                                else:
                                    nc.scalar.copy(
                                        ot[:, bi * w:(bi + 1) * w], ps[:])
                                if bi == 3:
                                    nc.scalar.dma_start(
                                        out2_r[:, 0:4, :], ot_h[:, 0:4, :])
                                elif bi == 6:
                                    nc.scalar.dma_start(
                                        out2_r[:, 4:7, :], ot_h[:, 4:7, :])
                        else:
                            if bi % 2 == 0:
                                nc.vector.tensor_copy(
                                    ot[:, bi * w:(bi + 1) * w], ps[:])
                            else:
                                nc.scalar.copy(
                                    ot[:, bi * w:(bi + 1) * w], ps[:])

                # writeback rides the scalar queue: it stays off the
                # sync queue (whose head-of-line order gates entity
                # prefetch) and off gpsimd (whose software DRAIN costs
                # ~2.3us + 29ns/DMA at teardown; the scalar drain is
                # ~0.5us). One coarse DMA per chunk; the last chunk is
                # handled above in bi-granular slices.
                if c < n_chunks - 1:
                    nc.scalar.dma_start(out_r[:, :, off:off + w],
                                        ot_h[:, :, :])
    nc.compile()
    return nc


def _prep_inputs(x, ent_emb, rel_emb, dt_name):
    x = np.asarray(x)
    ent_emb = np.asarray(ent_emb, dtype=np.float32)
    rel_emb = np.asarray(rel_emb, dtype=np.float32)
    r = ent_emb.shape[1] // 2
    lhs = ent_emb[x[:, 0]]
    rel = rel_emb[x[:, 1]]
    lre, lim = lhs[:, :r], lhs[:, r:]
    rre, rim = rel[:, :r], rel[:, r:]
    q = np.empty((x.shape[0], 2 * r), np.float32)
    q[:, :r] = lre * rre - lim * rim
    q[:, r:] = lre * rim + lim * rre

    if dt_name == "bf16":
        import ml_dtypes
        np_dt = ml_dtypes.bfloat16
    else:
        np_dt = np.float32

    qT = np.ascontiguousarray(q.T).astype(np_dt)           # [K, B]
    eT = np.ascontiguousarray(ent_emb.T).astype(np_dt)     # [K, N]
    in_maps = [
        {"qT": qT, "eT": np.ascontiguousarray(eT[:, i * NS:(i + 1) * NS])}
        for i in range(N_CORES)
    ]
    return in_maps


def run(x, ent_emb, rel_emb, dt_name=None, trace=False, **spmd_kwargs):
    dt_name = dt_name or os.environ.get("KERNEL_DT", "bf16")
    nc = build_kernel(dt_name)
    in_maps = _prep_inputs(x, ent_emb, rel_emb, dt_name)
    res = run_bass_kernel_spmd(
        nc, in_maps, list(range(N_CORES)), trace=trace, **spmd_kwargs
    )
    w_last = NS % CW if NS % CW else CW
    outs = []
    for i in range(N_CORES):
        main = np.asarray(res.results[i]["out"], dtype=np.float32)
        tail = np.asarray(res.results[i]["out2"], dtype=np.float32)
        # out2 is a raw [P, bt*w_last] tile dump; row bt*P+p of the
        # slab is tail[p, bt*w_last:(bt+1)*w_last]
        tail = tail.reshape(P, BT, w_last).transpose(1, 0, 2).reshape(
            B, w_last)
        outs.append(main)
        outs.append(tail)
    return np.concatenate(outs, axis=1), res


def kernel(x, ent_emb, rel_emb):
    out, _ = run(x, ent_emb, rel_emb)
    return out
